# revision 1
# baseline (speedup 1.0000x reference)
"""Trainium2 Bass kernel for int8-dequant Linear: out = x @ (W_q * scaler)^T.

Full shapes: x [4, 2048, 4096] f32, weight_q [4096, 4096] int8,
weight_scaler [4096] f32 -> out [4, 2048, 4096] f32.

Sharding: data-parallel over tokens (8192 tokens -> 1024 per core);
weight_q/scaler replicated. Each core computes out.T for its token
shard with out-channels on PSUM partitions; the per-channel scaler is
applied as a per-partition scalar multiply on PSUM eviction.

Matmul dtype: float32r (TF32-like fast fp32 mode, 1 cyc/row at free
dim >= 256; measured rel-err ~2e-4 on K=4096 accumulation).
Fallback MODE "bf16x2": W exact in bf16 (int8-valued), x split into
bf16 hi+lo, two accumulation passes (rel err ~5e-6, 2x matmul work).
"""
import sys

sys.path.insert(0, "/opt/trn_rl_repo")

import numpy as np

import concourse.bacc as bacc
import concourse.mybir as mybir
import concourse.tile as tile
from concourse.bass_utils import run_bass_kernel_spmd

N_CORES = 8
P = 128
IN_F = 4096
OUT_F = 4096
TOKENS = 4 * 2048
T_SHARD = TOKENS // N_CORES          # 1024 tokens per core
KT = IN_F // P                       # 32 k-tiles
MT = OUT_F // P                      # 32 m-tiles (out-channel tiles)
N_FREE = 512                         # moving free dim per matmul (1 PSUM bank)
NT = T_SHARD // N_FREE               # 2 n-tiles

MODE = "f32r"                        # "f32r" | "fp16" | "bf16x2"

_cache = {}


def _build(mode):
    f32 = mybir.dt.float32
    mm_dt = {
        "f32r": mybir.dt.float32r,
        "fp16": mybir.dt.float16,
        "bf16x2": mybir.dt.bfloat16,
    }[mode]
    n_pass = 2 if mode == "bf16x2" else 1

    nc = bacc.Bacc(None, target_bir_lowering=False, debug=False)

    # DRAM parameters (per-core shapes)
    d_x = [
        nc.declare_dram_parameter(f"xq{i}", [IN_F, T_SHARD], mm_dt, isOutput=False)
        for i in range(n_pass)
    ]
    # Weights travel as int8 (4x less DMA + in-window bandwidth) and are
    # upconverted to the matmul dtype on-chip by the otherwise-idle DVE.
    d_w = nc.declare_dram_parameter("wq4", [MT, P, KT, P], mybir.dt.int8, isOutput=False)
    d_s = nc.declare_dram_parameter("scal", [P, MT], f32, isOutput=False)
    d_o = nc.declare_dram_parameter("outT", [MT, P, T_SHARD], f32, isOutput=True)

    WH = 8                       # k-tiles per weight quarter-tile
    NH = KT // WH                # weight sub-tiles per mo
    PRO = 4 if n_pass == 1 else 0  # mo-tiles interleaved during the x load

    with tile.TileContext(nc) as tc:
        with (
            tc.tile_pool(name="xp", bufs=KT * n_pass) as xp,
            tc.tile_pool(name="wp", bufs=12) as wp,
            tc.tile_pool(name="ws", bufs=6) as ws,
            tc.tile_pool(name="op", bufs=4) as op,
            tc.tile_pool(name="cp", bufs=1) as cp,
            tc.tile_pool(name="ps", bufs=8, space="PSUM") as ps,
        ):
            scal = cp.tile([P, MT], f32)
            nc.sync.dma_start(scal[:], d_s.ap())

            def w_half(mo, h):
                s = ws.tile([P, WH, P], mybir.dt.int8, tag="w8", name=f"w8_{mo}_{h}")
                nc.sync.dma_start(s[:], d_w.ap()[mo, :, h * WH:(h + 1) * WH, :])
                t = wp.tile([P, WH, P], mm_dt, tag="wh", name=f"wh_{mo}_{h}")
                nc.vector.tensor_copy(t[:], s[:])
                return t

            def x_tile(i):
                ip, k = divmod(i, KT)
                t = xp.tile([P, T_SHARD], mm_dt, tag="xt", name=f"xt_{i}")
                nc.sync.dma_start(t[:], d_x[ip].ap()[k * P:(k + 1) * P, :])
                return t

            def evict(mo, n, psum):
                osb = op.tile([P, N_FREE], f32, tag="osb", name=f"osb_{mo}_{n}")
                nc.vector.tensor_scalar_mul(osb[:], psum[:], scal[:, mo:mo + 1])
                nc.sync.dma_start(
                    d_o.ap()[mo, :, n * N_FREE:(n + 1) * N_FREE], osb[:]
                )

            nk = KT * n_pass
            xt = [None] * nk
            wh_pro = {}

            # --- phase 1: first PRO mo-tiles, k-major across mo so the PE
            # has work for every x k-tile as it lands. Each k-group's
            # weight quarters are emitted just ahead of that group's x
            # k-tiles; the last group also prefetches mo=PRO's quarters so
            # phase 2 starts without waiting behind the x tail.
            wh_next = {}
            if PRO:
                for h in range(NH):
                    if h == 0:
                        # int8 quarters are tiny (128KB): emit all four
                        # before x0 so their DVE conversions complete during
                        # x0's transfer instead of serializing after it.
                        for mo in range(PRO):
                            wh_pro[(mo, 0)] = w_half(mo, 0)
                        for i in range(0, WH):
                            xt[i] = x_tile(i)
                    else:
                        for mo in range(PRO):
                            wh_pro[(mo, h)] = w_half(mo, h)
                        if h == NH - 1:
                            for j in range(NH):
                                wh_next[j] = w_half(PRO, j)
                        for i in range(h * WH, (h + 1) * WH):
                            xt[i] = x_tile(i)

                pro_ps = {
                    (mo, n): ps.tile(
                        [P, N_FREE], f32, tag="psum", name=f"psum_{mo}_{n}"
                    )
                    for mo in range(PRO)
                    for n in range(NT)
                }
                # Request mo=PRO's banks now so the allocator binds them to
                # the earliest-released phase-1 banks (shrinks the
                # phase-boundary stall).
                early_ps = [
                    ps.tile([P, N_FREE], f32, tag="psum", name=f"psum_{PRO}_{n}")
                    for n in range(NT)
                ]
                for i in range(nk):
                    ip, k = divmod(i, KT)
                    h, kh = divmod(k, WH)
                    for mo in range(PRO):
                        for n in range(NT):
                            nc.tensor.matmul(
                                pro_ps[(mo, n)][:],
                                wh_pro[(mo, h)][:, kh, :],
                                xt[i][:, n * N_FREE:(n + 1) * N_FREE],
                                start=(i == 0),
                                stop=(i == nk - 1),
                            )
                for mo in range(PRO):
                    for n in range(NT):
                        evict(mo, n, pro_ps[(mo, n)])
            else:
                early_ps = None
                for i in range(nk):
                    xt[i] = x_tile(i)

            # --- phase 2: remaining mo-tiles, weight-reuse-friendly order
            # (k middle, n inner).
            for mo in range(PRO, MT):
                if mo == PRO and wh_next:
                    whs = [wh_next[h] for h in range(NH)]
                else:
                    whs = [w_half(mo, h) for h in range(NH)]
                if mo == PRO and early_ps is not None:
                    psums = early_ps
                else:
                    psums = [
                        ps.tile([P, N_FREE], f32, tag="psum", name=f"psum_{mo}_{n}")
                        for n in range(NT)
                    ]
                for i in range(nk):
                    ip, k = divmod(i, KT)
                    h, kh = divmod(k, WH)
                    for n in range(NT):
                        nc.tensor.matmul(
                            psums[n][:],
                            whs[h][:, kh, :],
                            xt[i][:, n * N_FREE:(n + 1) * N_FREE],
                            start=(i == 0),
                            stop=(i == nk - 1),
                        )
                for n in range(NT):
                    evict(mo, n, psums[n])

    nc.compile()
    return nc


def _prep_inputs(x, weight_q, weight_scaler, mode):
    """Host-side shard + layout. Returns in_maps (list of dicts, one per core)."""
    xf = np.asarray(x, dtype=np.float32).reshape(TOKENS, IN_F)
    wq = np.asarray(weight_q)
    sc = np.asarray(weight_scaler, dtype=np.float32)

    # W tiles: w4[mo, p_in, ko, oc] = W[mo*128+oc, ko*128+p_in]
    # (matches the SBUF lhsT tile AP [P, KT, P] exactly), shipped as int8
    # and upconverted on-chip.
    w4 = np.ascontiguousarray(
        wq.reshape(MT, P, KT, P).transpose(0, 3, 2, 1)
    ).astype(np.int8)

    scal = np.ascontiguousarray(sc.reshape(MT, P).T)  # [P, MT]

    in_maps = []
    for c in range(N_CORES):
        xs = xf[c * T_SHARD:(c + 1) * T_SHARD, :]      # [T_SHARD, IN_F]
        xsT = np.ascontiguousarray(xs.T)                # [IN_F, T_SHARD] f32
        m = {"wq4": w4, "scal": scal}
        if mode == "f32r":
            m["xq0"] = xsT
        elif mode == "fp16":
            m["xq0"] = xsT.astype(np.float16)
        else:
            import ml_dtypes

            hi = xsT.astype(ml_dtypes.bfloat16)
            lo = (xsT - hi.astype(np.float32)).astype(ml_dtypes.bfloat16)
            m["xq0"] = hi
            m["xq1"] = lo
        in_maps.append(m)
    return in_maps


def _gather(results):
    """Per-core outT [MT, P, T_SHARD] -> full out [4, 2048, OUT_F] f32."""
    parts = []
    for c in range(N_CORES):
        ot = results[c]["outT"]                   # [MT, P, T_SHARD]
        parts.append(ot.reshape(OUT_F, T_SHARD).T)  # [T_SHARD, OUT_F]
    out = np.concatenate(parts, axis=0)           # [TOKENS, OUT_F]
    return np.ascontiguousarray(out.reshape(4, 2048, OUT_F), dtype=np.float32)


def _run(inputs, trace=False, mode=None):
    mode = mode or MODE
    if mode not in _cache:
        _cache[mode] = _build(mode)
    nc = _cache[mode]
    in_maps = _prep_inputs(inputs["x"], inputs["weight_q"], inputs["weight_scaler"], mode)
    res = run_bass_kernel_spmd(nc, in_maps, list(range(N_CORES)), trace=trace)
    return _gather(res.results), res


def kernel(**inputs):
    out, _ = _run(inputs, trace=False)
    return out



# revision 3
# speedup vs baseline: 1.0502x; 1.0502x over previous
"""Trainium2 Bass kernel for int8-dequant Linear: out = x @ (W_q * scaler)^T.

Full shapes: x [4, 2048, 4096] f32, weight_q [4096, 4096] int8,
weight_scaler [4096] f32 -> out [4, 2048, 4096] f32.

Sharding: data-parallel over tokens (8192 tokens -> 1024 per core);
weight_q/scaler replicated. Each core computes out.T for its token
shard with out-channels on PSUM partitions; the per-channel scaler is
applied as a per-partition scalar multiply on PSUM eviction.

Matmul dtype (MODE):
  "fp16"  - x and W both fp16. W int8-valued: exact (11-bit
            significand); x rounded to fp16: rel err ~5e-4, better
            than f32r's ~1.4e-3. 1 cyc/row on the PE, and the 2-byte
            LDWEIGHTS fully hides behind the 512-row matmul stream
            (f32r's 4-byte stationary load is ~187ns and stretches
            the steady-state period from 213ns to 227ns).
  "wbf16" - W bf16 stationary, x f32r moving (rel err ~1.4e-3).
  "f32r"  - both f32r (original baseline numerics).

Schedule notes:
  - DMA descriptor issue is ~0.65us serial per sequencer and programs
    land ~0.8us apart at the head, so x programs go on the Sync
    sequencer and weight/scaler/output programs on the Activation
    sequencer; neither stream queues behind the other.
  - PE warm-up dummy matmuls burn the 0.65->1.2->2.4GHz DVFS ramp
    while the first DMAs land (a >1us PE gap resets the ramp and
    costs ~9us of mid-clock matmuls).
  - Zero-stationary pad matmuls (accumulate +0 into the already-open
    PSUM group) absorb weight-cast arrival jitter at the k=0 seams.
  - The last mo-tile's eviction is chunked to shorten the tail.
"""
import sys

sys.path.insert(0, "/opt/trn_rl_repo")

import numpy as np

import concourse.bacc as bacc
import concourse.mybir as mybir
import concourse.tile as tile
from concourse.bass_utils import run_bass_kernel_spmd

N_CORES = 8
P = 128
IN_F = 4096
OUT_F = 4096
TOKENS = 4 * 2048
T_SHARD = TOKENS // N_CORES          # 1024 tokens per core
KT = IN_F // P                       # 32 k-tiles
MT = OUT_F // P                      # 32 m-tiles (out-channel tiles)
N_FREE = 512                         # moving free dim per matmul (1 PSUM bank)
NT = T_SHARD // N_FREE               # 2 n-tiles

MODE = "fp16"                        # "fp16" | "wbf16" | "bf16" | "f32r"

WARM_N = 8                           # PE warm-up dummy matmuls (512 rows each)
SEAM_PADS = 2                        # zero-matmul pads per k=0 mo-seam

_cache = {}


def _build(mode):
    f32 = mybir.dt.float32
    if mode == "fp16":
        x_dt = w_dt = mybir.dt.float16
    elif mode == "bf16":
        x_dt = w_dt = mybir.dt.bfloat16
    elif mode == "wbf16":
        x_dt, w_dt = mybir.dt.float32r, mybir.dt.bfloat16
    else:
        x_dt = w_dt = mybir.dt.float32r

    nc = bacc.Bacc(None, target_bir_lowering=False, debug=False)

    d_x = nc.declare_dram_parameter("xq0", [IN_F, T_SHARD], x_dt, isOutput=False)
    # Weights travel as int8 (4x less DMA) and are upconverted to the
    # stationary dtype on-chip by the otherwise-idle DVE.
    d_w = nc.declare_dram_parameter("wq4", [MT, P, KT, P], mybir.dt.int8, isOutput=False)
    d_s = nc.declare_dram_parameter("scal", [P, MT], f32, isOutput=False)
    d_o = nc.declare_dram_parameter("outT", [MT, P, T_SHARD], f32, isOutput=True)

    WH = 8                       # k-tiles per weight quarter-tile
    NH = KT // WH                # weight quarter-tiles per mo
    PRO = 4                      # mo-tiles interleaved during the x load

    with tile.TileContext(nc) as tc:
        with (
            tc.tile_pool(name="xh", bufs=NT) as xhp,
            tc.tile_pool(name="xp", bufs=KT - 1) as xp,
            tc.tile_pool(name="wp", bufs=12) as wp,
            tc.tile_pool(name="ws", bufs=8) as ws,
            tc.tile_pool(name="op", bufs=6) as op,
            tc.tile_pool(name="cp", bufs=3) as cp,
            tc.tile_pool(name="ps", bufs=8, space="PSUM") as ps,
        ):
            # PE warm-up / pad sources (memset on the idle DVE).
            warm = cp.tile([P, N_FREE], w_dt, name="warm")
            nc.vector.memset(warm[:], 1.0)
            zero_w = cp.tile([P, P], w_dt, name="zero_w")
            nc.vector.memset(zero_w[:], 0.0)

            def w_half(mo, h):
                s = ws.tile([P, WH, P], mybir.dt.int8, tag="w8", name=f"w8_{mo}_{h}")
                nc.scalar.dma_start(s[:], d_w.ap()[mo, :, h * WH:(h + 1) * WH, :])
                t = wp.tile([P, WH, P], w_dt, tag="wh", name=f"wh_{mo}_{h}")
                nc.vector.tensor_copy(t[:], s[:])
                return t

            xht = {}                 # n -> [P, N_FREE] half tile for k=0
            xt = [None] * KT         # k -> [P, T_SHARD] tile, k >= 1

            def x_half(n):
                t = xhp.tile([P, N_FREE], x_dt, tag="xht", name=f"xh_0_{n}")
                nc.sync.dma_start(
                    t[:], d_x.ap()[0:P, n * N_FREE:(n + 1) * N_FREE]
                )
                xht[n] = t

            def x_tile(k):
                t = xp.tile([P, T_SHARD], x_dt, tag="xt", name=f"xt_{k}")
                nc.sync.dma_start(t[:], d_x.ap()[k * P:(k + 1) * P, :])
                xt[k] = t

            def x_slice(k, n):
                if k == 0:
                    return xht[n][:]
                return xt[k][:, n * N_FREE:(n + 1) * N_FREE]

            def evict(mo, n, psum, scal, chunks=1):
                cw = N_FREE // chunks
                for c in range(chunks):
                    osb = op.tile([P, cw], f32, tag="osb", name=f"osb_{mo}_{n}_{c}")
                    nc.vector.tensor_scalar_mul(
                        osb[:], psum[:, c * cw:(c + 1) * cw], scal[:, mo:mo + 1]
                    )
                    nc.scalar.dma_start(
                        d_o.ap()[mo, :, n * N_FREE + c * cw:n * N_FREE + (c + 1) * cw],
                        osb[:],
                    )

            # --- head DMA issues.  Weight quarters (h=0) stream on the
            # Activation sequencer while x streams on Sync.
            wh_pro = {}
            for mo in range(PRO):
                wh_pro[(mo, 0)] = w_half(mo, 0)
            x_half(0)
            x_half(1)
            for k in range(1, WH):
                x_tile(k)
            scal = cp.tile([P, MT], f32, name="scal")
            nc.scalar.dma_start(scal[:], d_s.ap())

            # remaining k-groups: quarter weights just ahead of their x
            # tiles; the last group also prefetches mo=PRO's quarters so
            # phase 2 starts without waiting behind the x tail.
            wh_next = {}
            for h in range(1, NH):
                for mo in range(PRO):
                    wh_pro[(mo, h)] = w_half(mo, h)
                if h == NH - 1:
                    for j in range(NH):
                        wh_next[j] = w_half(PRO, j)
                for k in range(h * WH, (h + 1) * WH):
                    x_tile(k)

            # --- PE warm-up: dummy matmuls with no DMA deps keep the
            # clock ramping until the first weight cast lands.
            warm_ps = ps.tile([P, N_FREE], f32, tag="psum", name="psum_warm")
            for i in range(WARM_N):
                nc.tensor.matmul(
                    warm_ps[:], warm[:, 0:P], warm[:], start=True, stop=True
                )

            # --- phase 1 matmuls: first PRO mo-tiles, k-major so the PE
            # has work for every x k-tile as it lands.
            pro_ps = {
                (mo, n): ps.tile([P, N_FREE], f32, tag="psum", name=f"psum_{mo}_{n}")
                for mo in range(PRO)
                for n in range(NT)
            }
            # Request mo=PRO's banks now so the allocator binds them to
            # the earliest-released phase-1 banks.
            early_ps = [
                ps.tile([P, N_FREE], f32, tag="psum", name=f"psum_{PRO}_{n}")
                for n in range(NT)
            ]
            for k in range(KT):
                h, kh = divmod(k, WH)
                for mo in range(PRO):
                    if k == 0 and mo > 0:
                        # +0 pads into mo-1's open group absorb cast
                        # arrival jitter without idling the PE.
                        for _ in range(SEAM_PADS):
                            nc.tensor.matmul(
                                pro_ps[(mo - 1, 0)][:],
                                zero_w[:],
                                warm[:],
                                start=False,
                                stop=False,
                            )
                    for n in range(NT):
                        nc.tensor.matmul(
                            pro_ps[(mo, n)][:],
                            wh_pro[(mo, h)][:, kh, :],
                            x_slice(k, n),
                            start=(k == 0),
                            stop=(k == KT - 1),
                        )
            for mo in range(PRO):
                for n in range(NT):
                    evict(mo, n, pro_ps[(mo, n)], scal)

            # --- phase 2: remaining mo-tiles, weight-reuse order.
            for mo in range(PRO, MT):
                if mo == PRO:
                    whs = [wh_next[h] for h in range(NH)]
                    psums = early_ps
                else:
                    whs = [w_half(mo, h) for h in range(NH)]
                    psums = [
                        ps.tile([P, N_FREE], f32, tag="psum", name=f"psum_{mo}_{n}")
                        for n in range(NT)
                    ]
                for k in range(KT):
                    h, kh = divmod(k, WH)
                    for n in range(NT):
                        nc.tensor.matmul(
                            psums[n][:],
                            whs[h][:, kh, :],
                            x_slice(k, n),
                            start=(k == 0),
                            stop=(k == KT - 1),
                        )
                chunks = 2 if mo == MT - 1 else 1
                for n in range(NT):
                    evict(mo, n, psums[n], scal, chunks=chunks)

    nc.compile()
    return nc


def _prep_inputs(x, weight_q, weight_scaler, mode):
    """Host-side shard + layout. Returns in_maps (list of dicts, one per core)."""
    xf = np.asarray(x, dtype=np.float32).reshape(TOKENS, IN_F)
    wq = np.asarray(weight_q)
    sc = np.asarray(weight_scaler, dtype=np.float32)

    # W tiles: w4[mo, p_in, ko, oc] = W[mo*128+oc, ko*128+p_in]
    # (matches the SBUF lhsT tile AP [P, KT, P] exactly), shipped as int8
    # and upconverted on-chip.
    w4 = np.ascontiguousarray(
        wq.reshape(MT, P, KT, P).transpose(0, 3, 2, 1)
    ).astype(np.int8)

    scal = np.ascontiguousarray(sc.reshape(MT, P).T)  # [P, MT]

    if mode == "fp16":
        x_cast = lambda a: a.astype(np.float16)
    elif mode == "bf16":
        import ml_dtypes

        x_cast = lambda a: a.astype(ml_dtypes.bfloat16)
    else:
        x_cast = lambda a: a

    in_maps = []
    for c in range(N_CORES):
        xs = xf[c * T_SHARD:(c + 1) * T_SHARD, :]      # [T_SHARD, IN_F]
        xsT = np.ascontiguousarray(xs.T)                # [IN_F, T_SHARD] f32
        in_maps.append({"wq4": w4, "scal": scal, "xq0": x_cast(xsT)})
    return in_maps


def _gather(results):
    """Per-core outT [MT, P, T_SHARD] -> full out [4, 2048, OUT_F] f32."""
    parts = []
    for c in range(N_CORES):
        ot = results[c]["outT"]                   # [MT, P, T_SHARD]
        parts.append(ot.reshape(OUT_F, T_SHARD).T)  # [T_SHARD, OUT_F]
    out = np.concatenate(parts, axis=0)           # [TOKENS, OUT_F]
    return np.ascontiguousarray(out.reshape(4, 2048, OUT_F), dtype=np.float32)


def _run(inputs, trace=False, mode=None):
    mode = mode or MODE
    if mode not in _cache:
        _cache[mode] = _build(mode)
    nc = _cache[mode]
    in_maps = _prep_inputs(inputs["x"], inputs["weight_q"], inputs["weight_scaler"], mode)
    res = run_bass_kernel_spmd(nc, in_maps, list(range(N_CORES)), trace=trace)
    return _gather(res.results), res


def kernel(**inputs):
    out, _ = _run(inputs, trace=False)
    return out
